# revision 1
# baseline (speedup 1.0000x reference)
"""GCN classifier (2x GCNConv + mean-pool + linear) on 8 Trainium2 NeuronCores.

Strategy:
  - Destination-node sharding: core c owns nodes [6250c, 6250(c+1)).
  - Self-loops appended as explicit edges; edges sorted by (dest block,
    table half, position).
  - y1 = dinv * (x @ W1) computed replicated on every core (bf16 table).
  - Aggregation: per (dest block, table half), one custom dma_gather pulls all
    source rows (bf16, 256B rows) into chunk layout; per 128-edge chunk a
    one-hot selection matrix (DVE is_equal, bf16) and a PE matmul accumulate
    into f32 PSUM.
  - h1 -> xw2 -> y2 slice per dest block; y2 slices exchanged via chunked
    AllGather collectives overlapped with layer-1 aggregation.
  - Mean-pool via selection matmuls into persistent PSUM accumulators,
    AllReduce of per-graph partial sums/counts, final linear on every core.
"""
import numpy as np

import concourse.bacc as bacc
import concourse.bass as bass
import concourse.mybir as mybir
import concourse.tile as tile
from concourse import bass_utils

# problem dims (hardcoded per contract)
N = 50000
E = 600000
IN_CH = 256
HID = 128
NUM_CLASSES = 2
G = 256
NCORES = 8

SLICE = N // NCORES          # 6250 nodes per core
NB = (SLICE + 127) // 128    # 49 dest blocks per core
PB = NB * 128                # 6272 padded rows per core
PADN = PB * NCORES           # 50176 padded table rows
HALF = PADN // 2             # 25088 rows per gather-table half (int16 range)
CHUNKS = [7, 7, 7, 4, 3, 7, 7, 7]   # dest blocks per allgather chunk
HALF2 = 25600                # layer-2 half boundary (chunks 0-3)
_SB = np.concatenate([[0], np.cumsum(CHUNKS)])       # chunk -> first block
_CO = np.concatenate([[0], np.cumsum([8 * 128 * w for w in CHUNKS])])  # chunk -> table row
_J_OF_B = np.repeat(np.arange(len(CHUNKS)), CHUNKS)  # block -> chunk
MAXC = 8                     # dma_gather cap: num_idxs <= 1024

dt = mybir.dt

_cache = {}


def _pos1(u):
    """node id -> row in core-major padded table (y1 layout)."""
    return PB * (u // SLICE) + (u % SLICE)


def _pos2(u):
    """node id -> row in chunk-major padded table (y2 allgather layout)."""
    c = u // SLICE
    l = u % SLICE
    b = l // 128
    p = l % 128
    j = _J_OF_B[b]
    w = np.asarray(CHUNKS)[j]
    return _CO[j] + c * (w * 128) + (b - _SB[j]) * 128 + p


def _wrap_idx(flat):
    """edge-slot-ordered positions [n] -> dma_gather wrapped layout [128, n//16]."""
    n = flat.shape[0]
    cols = n // 16
    out = np.empty((128, cols), np.int16)
    block = flat.reshape(cols, 16).T  # [16, cols]
    for g in range(8):
        out[g * 16 : (g + 1) * 16] = block
    return out


def _host_prep(x, edge_index, batch):
    x = np.asarray(x, np.float32)
    ei = np.asarray(edge_index)
    batch_np = np.asarray(batch)

    src = np.concatenate([ei[0], np.arange(N, dtype=ei.dtype)]).astype(np.int64)
    dst = np.concatenate([ei[1], np.arange(N, dtype=ei.dtype)]).astype(np.int64)
    deg = np.bincount(dst, minlength=N).astype(np.float32)  # >= 1 (self-loops)

    # per-core edges with per-layer (block, half) grouping
    layers = {1: _pos1, 2: _pos2}
    ecore = {}
    counts = {l: np.zeros((NCORES, NB, 2), np.int64) for l in layers}
    for c in range(NCORES):
        m = (dst // SLICE) == c
        es, ed = src[m], dst[m]
        ld = ed - SLICE * c
        ecore[c] = {}
        for l, posf in layers.items():
            hb = HALF if l == 1 else HALF2
            pos = posf(es)
            half = (pos >= hb).astype(np.int64)
            order = np.lexsort((pos, half, ld // 128))
            p_s, h_s, ld_s = pos[order], half[order], ld[order]
            b_s = ld_s // 128
            for b in range(NB):
                for h in (0, 1):
                    sel = (b_s == b) & (h_s == h)
                    counts[l][c, b, h] = sel.sum()
            ecore[c][l] = (p_s, h_s, b_s, ld_s % 128)

    # SPMD-uniform gather sizes per (layer, block, half): %16 granular
    NI2 = {}
    K2 = {}
    for l in layers:
        mx = counts[l].max(axis=0)  # [NB, 2]
        ni = np.maximum((mx + 15) // 16 * 16, 16)
        assert ni.max() <= 1024, ni.max()
        NI2[l] = ni.astype(np.int64)
        K2[l] = ((ni + 127) // 128).astype(np.int64)

    idx_np = {}
    colloc_np = {}
    for l in layers:
        C2 = int(K2[l].sum())
        ICOLS = int(NI2[l].sum()) // 16
        idx_np[l] = np.zeros((NCORES, 128, ICOLS), np.int16)
        colloc_np[l] = np.full((NCORES, 128, C2), -1.0, np.float32)
        hb = HALF if l == 1 else HALF2
        for c in range(NCORES):
            p_s, h_s, b_s, lp_s = ecore[c][l]
            col = 0
            icol = 0
            for h in (0, 1):
                for b in range(NB):
                    K = int(K2[l][b, h])
                    ni = int(NI2[l][b, h])
                    sel = (b_s == b) & (h_s == h)
                    k = int(sel.sum())
                    p_pad = np.zeros(ni, np.int64)
                    c_pad = np.full(K * 128, -1.0, np.float32)
                    p_pad[:k] = p_s[sel] - h * hb
                    c_pad[:k] = lp_s[sel]
                    idx_np[l][c, :, icol : icol + ni // 16] = _wrap_idx(
                        p_pad.astype(np.int16)
                    )
                    colloc_np[l][c, :, col : col + K] = c_pad.reshape(K, 128).T
                    col += K
                    icol += ni // 16
        colloc_np[l] = colloc_np[l].astype(np.float32)

    # degrees in block layout
    degp_slices = []
    degs_core = []
    batch_core = []
    for c in range(NCORES):
        dpad = np.ones(PB, np.float32)
        dpad[:SLICE] = deg[c * SLICE : (c + 1) * SLICE]
        degp_slices.append(dpad)
        degs_core.append(dpad.reshape(NB, 128).T.copy())
        bpad = np.full(PB, -1.0, np.float32)
        bpad[:SLICE] = batch_np[c * SLICE : (c + 1) * SLICE].astype(np.float32)
        batch_core.append(bpad.reshape(NB, 128).T.copy())
    degp = np.concatenate(degp_slices).reshape(NCORES * NB, 128).T.copy()

    # padded x (row-major, layer-1 gather table) + per-edge dinv[src]
    xp = np.zeros((PADN, IN_CH), np.float32)
    xp[_pos1(np.arange(N))] = x
    dinv_all = 1.0 / np.sqrt(deg)
    dsrc_np = np.zeros((NCORES, 128, int(K2[1].sum())), np.float32)
    for c in range(NCORES):
        p_s, h_s, b_s, lp_s = ecore[c][1]
        col = 0
        for h in (0, 1):
            for b in range(NB):
                K = int(K2[1][b, h])
                sel = (b_s == b) & (h_s == h)
                k = int(sel.sum())
                d_pad = np.zeros(K * 128, np.float32)
                pv = p_s[sel]
                d_pad[:k] = dinv_all[(pv // PB) * SLICE + (pv % PB)]
                dsrc_np[c][:, col : col + K] = d_pad.reshape(K, 128).T
                col += K
    return {
        "xp": xp,
        "dsrc1": dsrc_np,
        "xT": np.ascontiguousarray(xp.T),
        "degp": degp,
        "degs": degs_core,
        "batch": batch_core,
        "idx": idx_np,
        "colloc": colloc_np,
        "K2": K2,
        "NI2": NI2,
    }


def _build_program(K2, NI2):
    nc = bacc.Bacc("TRN2", target_bir_lowering=False, debug=False, num_devices=NCORES)
    f32, bf16, i16 = dt.float32, dt.bfloat16, dt.int16
    C2 = {l: int(K2[l].sum()) for l in (1, 2)}
    IC = {l: int(NI2[l].sum()) // 16 for l in (1, 2)}

    # ---- I/O ----
    xp_d = nc.dram_tensor("xp", [PADN, IN_CH], bf16, kind="ExternalInput")
    dsrc1_d = nc.dram_tensor("dsrc1", [128, C2[1]], f32, kind="ExternalInput")
    w1_d = nc.dram_tensor("W1", [IN_CH, HID], bf16, kind="ExternalInput")
    w2_d = nc.dram_tensor("W2", [HID, HID], bf16, kind="ExternalInput")
    b1r_d = nc.dram_tensor("b1r", [128, HID], f32, kind="ExternalInput")
    b2r_d = nc.dram_tensor("b2r", [128, HID], f32, kind="ExternalInput")
    linw_d = nc.dram_tensor("linW", [HID, NUM_CLASSES], bf16, kind="ExternalInput")
    linbr_d = nc.dram_tensor("linbr", [128, NUM_CLASSES], f32, kind="ExternalInput")
    iota128_d = nc.dram_tensor("iota128", [128, 128], bf16, kind="ExternalInput")
    iota256_d = nc.dram_tensor("iota256", [128, G], bf16, kind="ExternalInput")
    identb_d = nc.dram_tensor("identb", [128, 128], bf16, kind="ExternalInput")
    degp_d = nc.dram_tensor("degp", [128, NCORES * NB], f32, kind="ExternalInput")
    degs_d = nc.dram_tensor("degs", [128, NB], f32, kind="ExternalInput")
    batch_d = nc.dram_tensor("batch", [128, NB], bf16, kind="ExternalInput")
    idx1_d = nc.dram_tensor("idx1", [128, IC[1]], i16, kind="ExternalInput")
    idx2_d = nc.dram_tensor("idx2", [128, IC[2]], i16, kind="ExternalInput")
    col1_d = nc.dram_tensor("col1", [128, C2[1]], bf16, kind="ExternalInput")
    col2_d = nc.dram_tensor("col2", [128, C2[2]], bf16, kind="ExternalInput")
    out_d = nc.dram_tensor("out", [G, NUM_CLASSES], f32, kind="ExternalOutput")

    # ---- internal DRAM ----
    y2slice = nc.dram_tensor("y2slice", [PB, HID], bf16, kind="Internal")
    y2a = nc.dram_tensor("y2a", [HALF2, HID], bf16, kind="Internal", addr_space="Shared")
    y2b = nc.dram_tensor("y2b", [PADN - HALF2, HID], bf16, kind="Internal", addr_space="Shared")
    pool_in = nc.dram_tensor("pool_in", [G, HID + 1], f32, kind="Internal")
    pool_out = nc.dram_tensor(
        "pool_out", [G, HID + 1], f32, kind="Internal", addr_space="Shared"
    )
    dum_d = nc.dram_tensor("dumt", [16, HID], bf16, kind="ExternalInput")

    AG_GROUPS = [list(range(NCORES))]

    with tile.TileContext(nc) as tc:
        with (
            tc.tile_pool(name="consts", bufs=1) as cons,
            tc.tile_pool(name="xtiles", bufs=3) as xtl,
            tc.tile_pool(name="work", bufs=4) as work,
            tc.tile_pool(name="idxt", bufs=4) as idxt,
            tc.tile_pool(name="msgs", bufs=12) as msgs,
            tc.tile_pool(name="sels", bufs=32) as sels,
            tc.tile_pool(name="spill", bufs=1) as spl,
            tc.tile_pool(name="psum", bufs=2, space="PSUM") as pst,
            tc.tile_pool(name="psumx", bufs=1, space="PSUM") as psx,
            tc.tile_pool(name="psump", bufs=1, space="PSUM") as psp,
        ):
            # ---- constants ----
            w1a = cons.tile([128, HID], bf16, tag="w1a")
            w1b = cons.tile([128, HID], bf16, tag="w1b")
            w2 = cons.tile([128, HID], bf16, tag="w2")
            b1r = cons.tile([128, HID], f32, tag="b1r")
            b2r = cons.tile([128, HID], f32, tag="b2r")
            linw = cons.tile([HID, NUM_CLASSES], bf16, tag="linw")
            linbr = cons.tile([128, NUM_CLASSES], f32, tag="linbr")
            iota128 = cons.tile([128, 128], bf16, tag="iota128")
            iota256 = cons.tile([128, G], bf16, tag="iota256")
            identb = cons.tile([128, 128], bf16, tag="identb")
            degp = cons.tile([128, NCORES * NB], f32, tag="degp")
            degs = cons.tile([128, NB], f32, tag="degs")
            batcht = cons.tile([128, NB], bf16, tag="batcht")
            col1 = cons.tile([128, C2[1]], bf16, tag="col1")
            col2 = cons.tile([128, C2[2]], bf16, tag="col2")
            dsrc1 = cons.tile([128, C2[1]], f32, tag="dsrc1")
            idx1 = cons.tile([128, IC[1]], i16, tag="idx1")
            idx2 = cons.tile([128, IC[2]], i16, tag="idx2")
            dinv_g = cons.tile([128, NCORES * NB], f32, tag="dinv_g")
            dinv_s = cons.tile([128, NB], f32, tag="dinv_s")

            for t, d in (
                (w1a, w1_d[0:128, :]), (w1b, w1_d[128:256, :]), (w2, w2_d[:]),
                (b1r, b1r_d[:]), (b2r, b2r_d[:]), (linw, linw_d[:]),
                (linbr, linbr_d[:]), (iota128, iota128_d[:]),
                (iota256, iota256_d[:]), (identb, identb_d[:]),
                (degp, degp_d[:]), (degs, degs_d[:]), (batcht, batch_d[:]),
                (col1, col1_d[:]), (col2, col2_d[:]),
                (idx1, idx1_d[:]), (idx2, idx2_d[:]), (dsrc1, dsrc1_d[:]),
            ):
                nc.sync.dma_start(t[:], d)

            # tiny dummy gather up front so the GPSIMD "mlp" library reload
            # overlaps phase A instead of stalling the first real gather
            dumi = cons.tile([128, 1], i16, tag="dumi")
            nc.vector.memset(dumi[:], 0)
            dumm = cons.tile([128, HID], bf16, tag="dumm")
            nc.gpsimd.dma_gather(
                out_ap=dumm[:].rearrange("p (c e) -> p c e", e=HID),
                in_ap=dum_d[:],
                idxs_ap=dumi[:],
                num_idxs=16,
                num_idxs_reg=16,
                elem_size=HID,
            )

            # dinv = 1/sqrt(deg)
            nc.scalar.activation(dinv_g[:], degp[:], mybir.ActivationFunctionType.Sqrt)
            nc.vector.reciprocal(dinv_g[:], dinv_g[:])
            nc.scalar.activation(dinv_s[:], degs[:], mybir.ActivationFunctionType.Sqrt)
            nc.vector.reciprocal(dinv_s[:], dinv_s[:])

            # ---- phase A: y1 = dinv * (x @ W1), full table, replicated ----


            # warm all msg pool slots so never-gathered tail rows are finite
            for _ in range(12):
                wt = msgs.tile([128, MAXC * IN_CH], bf16, tag="msg")
                nc.vector.memset(wt[:], 0.0)

            # ---- phases B (layer 1) and C (layer 2) ----
            for layer in (1, 2):
                K = K2[layer]
                NI = NI2[layer]
                idxt_t = idx1 if layer == 1 else idx2
                coll = col1 if layer == 1 else col2
                EW = IN_CH if layer == 1 else HID
                tables = (None, None) if layer == 1 else (y2a, y2b)
                brep = b1r if layer == 1 else b2r
                ibase = 0
                cbase = 0
                spills = {}
                for h in (0, 1):
                  table_h = tables[h]
                  for b in range(NB):
                    Kh = int(K[b, h])
                    ni = int(NI[b, h])
                    ps = pst.tile([128, EW], f32, tag="agg", space="PSUM")
                    mt = msgs.tile([128, MAXC * IN_CH], bf16, tag="msg")
                    nc.gpsimd.dma_gather(
                        out_ap=mt[:, : Kh * EW].rearrange("p (c e) -> p c e", e=EW),
                        in_ap=(xp_d[h * HALF : (h + 1) * HALF, :] if layer == 1 else table_h[:]),
                        idxs_ap=idxt_t[:, ibase : ibase + ni // 16],
                        num_idxs=ni,
                        num_idxs_reg=ni,
                        elem_size=EW,
                    )
                    for j in range(Kh):
                        S = sels.tile([128, 128], bf16, tag="sel")
                        nc.vector.tensor_tensor(
                            out=S[:],
                            in0=coll[:, cbase + j : cbase + j + 1].to_broadcast(
                                [128, 128]
                            ),
                            in1=iota128[:],
                            op=mybir.AluOpType.is_equal,
                        )
                        if layer == 1:
                            nc.vector.tensor_tensor(
                                out=S[:],
                                in0=S[:],
                                in1=dsrc1[:, cbase + j : cbase + j + 1].to_broadcast(
                                    [128, 128]
                                ),
                                op=mybir.AluOpType.mult,
                            )
                        nc.tensor.matmul(
                            ps[:],
                            S[:],
                            mt[:, j * EW : (j + 1) * EW],
                            start=(j == 0),
                            stop=(j == Kh - 1),
                        )
                    ibase += ni // 16
                    cbase += Kh
                    if h == 0:
                        sp = spl.tile([128, EW], f32, tag=f"sp{b}")
                        nc.vector.tensor_copy(out=sp[:], in_=ps[:])
                        spills[b] = sp
                        continue
                    # epilogue: h = relu(dinv * (agg0 + agg1)[@W1] + b)
                    if layer == 1:
                        axf = work.tile([128, IN_CH], bf16, tag="axf")
                        nc.vector.tensor_tensor(
                            out=axf[:], in0=ps[:], in1=spills[b][:],
                            op=mybir.AluOpType.add,
                        )
                        agg1 = psx.tile([128, HID], f32, tag="xw1o", space="PSUM")
                        for q in range(2):
                            ptq = psx.tile([128, 128], bf16, tag="trans", space="PSUM")
                            nc.tensor.transpose(
                                out=ptq[:], in_=axf[:, q * 128 : (q + 1) * 128],
                                identity=identb[:],
                            )
                            tq = work.tile([128, 128], bf16, tag="tq")
                            nc.vector.tensor_copy(out=tq[:], in_=ptq[:])
                            nc.tensor.matmul(
                                agg1[:], tq[:], w1a[:] if q == 0 else w1b[:],
                                start=(q == 0), stop=(q == 1),
                            )
                        accv = agg1
                    else:
                        accv = work.tile([128, HID], f32, tag="hf0")
                        nc.vector.tensor_tensor(
                            out=accv[:], in0=ps[:], in1=spills[b][:],
                            op=mybir.AluOpType.add,
                        )
                    hf = work.tile([128, HID], f32, tag="hf")
                    nc.vector.tensor_tensor(
                        out=hf[:],
                        in0=accv[:],
                        in1=dinv_s[:, b : b + 1].to_broadcast([128, HID]),
                        op=mybir.AluOpType.mult,
                    )
                    nc.vector.tensor_tensor(
                        out=hf[:], in0=hf[:], in1=brep[:], op=mybir.AluOpType.add
                    )
                    hb = work.tile([128, HID], bf16, tag="hb")
                    nc.scalar.activation(
                        hb[:], hf[:], mybir.ActivationFunctionType.Relu
                    )
                    if layer == 1:
                        # xw2 = h1 @ W2 ; y2 = dinv * xw2
                        pt = psx.tile([128, 128], bf16, tag="trans", space="PSUM")
                        nc.tensor.transpose(out=pt[:], in_=hb[:], identity=identb[:])
                        hT = work.tile([128, 128], bf16, tag="hT")
                        nc.vector.tensor_copy(out=hT[:], in_=pt[:])
                        p2 = psx.tile([128, HID], f32, tag="xw2", space="PSUM")
                        nc.tensor.matmul(p2[:], hT[:], w2[:], start=True, stop=True)
                        y2w = work.tile([128, HID], bf16, tag="y2w")
                        nc.vector.tensor_tensor(
                            out=y2w[:],
                            in0=p2[:],
                            in1=dinv_s[:, b : b + 1].to_broadcast([128, HID]),
                            op=mybir.AluOpType.mult,
                        )
                        nc.sync.dma_start(y2slice[b * 128 : (b + 1) * 128, :], y2w[:])
                        if b + 1 in _SB[1:]:
                            j = int(np.searchsorted(_SB, b + 1)) - 1
                            r0 = int(_SB[j]) * 128
                            rn = CHUNKS[j] * 128
                            o0 = int(_CO[j])
                            ytgt, yo = (y2a, o0) if j <= 3 else (y2b, o0 - HALF2)
                            nc.gpsimd.collective_compute(
                                "AllGather",
                                mybir.AluOpType.bypass,
                                replica_groups=AG_GROUPS,
                                ins=[y2slice[r0 : r0 + rn, :].opt()],
                                outs=[ytgt[yo : yo + NCORES * rn, :].opt()],
                            )
                    else:
                        # pooling: append ones column, selection matmuls
                        ho = work.tile([128, HID + 1], bf16, tag="hones")
                        nc.vector.tensor_copy(out=ho[:, :HID], in_=hb[:])
                        nc.vector.memset(ho[:, HID : HID + 1], 1.0)
                        Sp = work.tile([128, G], bf16, tag="spool")
                        nc.vector.tensor_tensor(
                            out=Sp[:],
                            in0=batcht[:, b : b + 1].to_broadcast([128, G]),
                            in1=iota256[:],
                            op=mybir.AluOpType.is_equal,
                        )
                        if b == 0:
                            ppA = psp.tile([128, HID + 1], f32, tag="poolA", space="PSUM")
                            ppB = psp.tile([128, HID + 1], f32, tag="poolB", space="PSUM")
                        nc.tensor.matmul(
                            ppA[:], Sp[:, 0:128], ho[:], start=(b == 0), stop=(b == NB - 1)
                        )
                        nc.tensor.matmul(
                            ppB[:], Sp[:, 128:256], ho[:], start=(b == 0), stop=(b == NB - 1)
                        )

            # ---- phase D: reduce partial sums, final linear ----
            sA = work.tile([128, HID + 1], f32, tag="sA")
            sB = work.tile([128, HID + 1], f32, tag="sB")
            nc.vector.tensor_copy(out=sA[:], in_=ppA[:])
            nc.vector.tensor_copy(out=sB[:], in_=ppB[:])
            nc.sync.dma_start(pool_in[0:128, :], sA[:])
            nc.sync.dma_start(pool_in[128:256, :], sB[:])
            nc.gpsimd.collective_compute(
                "AllReduce",
                mybir.AluOpType.add,
                replica_groups=AG_GROUPS,
                ins=[pool_in[:].opt()],
                outs=[pool_out[:].opt()],
            )
            for half in range(2):
                s = work.tile([128, HID + 1], f32, tag="sred")
                nc.sync.dma_start(s[:], pool_out[half * 128 : (half + 1) * 128, :])
                cnt = work.tile([128, 1], f32, tag="cnt")
                nc.vector.tensor_scalar_max(cnt[:], s[:, HID : HID + 1], 1.0)
                rc = work.tile([128, 1], f32, tag="rc")
                nc.vector.reciprocal(rc[:], cnt[:])
                sbt = work.tile([128, HID], bf16, tag="sbt")
                nc.vector.tensor_copy(out=sbt[:], in_=s[:, 0:HID])
                pt = psx.tile([128, 128], bf16, tag="trans", space="PSUM")
                nc.tensor.transpose(out=pt[:], in_=sbt[:], identity=identb[:])
                sT = work.tile([128, 128], bf16, tag="sT")
                nc.vector.tensor_copy(out=sT[:], in_=pt[:])
                po = psx.tile([128, NUM_CLASSES], f32, tag="xw1o", space="PSUM")
                nc.tensor.matmul(po[:], sT[:], linw[:], start=True, stop=True)
                ob = work.tile([128, NUM_CLASSES], f32, tag="ob")
                nc.vector.tensor_tensor(
                    out=ob[:],
                    in0=po[:],
                    in1=rc[:].to_broadcast([128, NUM_CLASSES]),
                    op=mybir.AluOpType.mult,
                )
                nc.vector.tensor_tensor(
                    out=ob[:], in0=ob[:], in1=linbr[:], op=mybir.AluOpType.add
                )
                nc.sync.dma_start(out_d[half * 128 : (half + 1) * 128, :], ob[:])

    nc.compile()
    return nc


def _get_program(prep):
    key = tuple(tuple(map(tuple, prep["NI2"][l])) for l in (1, 2))
    if key not in _cache:
        _cache[key] = _build_program(prep["K2"], prep["NI2"])
    return _cache[key]


def _to_bf16(a):
    import ml_dtypes

    return np.asarray(a, np.float32).astype(ml_dtypes.bfloat16)


def _run(x, edge_index, batch, W1, b1, W2, b2, lin_W, lin_b, trace=False):
    prep = _host_prep(x, edge_index, batch)
    nc = _get_program(prep)

    b1r = np.tile(np.asarray(b1, np.float32)[None, :], (128, 1))
    b2r = np.tile(np.asarray(b2, np.float32)[None, :], (128, 1))
    linbr = np.tile(np.asarray(lin_b, np.float32)[None, :], (128, 1))
    iota128 = _to_bf16(np.tile(np.arange(128, dtype=np.float32)[None, :], (128, 1)))
    iota256 = _to_bf16(np.tile(np.arange(G, dtype=np.float32)[None, :], (128, 1)))
    identb = _to_bf16(np.eye(128, dtype=np.float32))
    xTb = _to_bf16(prep["xT"])
    xpb = _to_bf16(prep["xp"])
    W1b = _to_bf16(W1)
    W2b = _to_bf16(W2)
    linwb = _to_bf16(lin_W)

    in_maps = []
    for c in range(NCORES):
        in_maps.append(
            {
                "xp": xpb,
                "dsrc1": prep["dsrc1"][c],
                "xT": xTb,
                "W1": W1b,
                "W2": W2b,
                "b1r": b1r,
                "b2r": b2r,
                "linW": linwb,
                "linbr": linbr,
                "iota128": iota128,
                "iota256": iota256,
                "identb": identb,
                "degp": prep["degp"],
                "degs": prep["degs"][c],
                "batch": _to_bf16(prep["batch"][c]),
                "dumt": _to_bf16(np.zeros((16, HID), np.float32)),
                "idx1": np.ascontiguousarray(prep["idx"][1][c]),
                "idx2": np.ascontiguousarray(prep["idx"][2][c]),
                "col1": _to_bf16(prep["colloc"][1][c]),
                "col2": _to_bf16(prep["colloc"][2][c]),
            }
        )

    res = bass_utils.run_bass_kernel_spmd(
        nc, in_maps, core_ids=list(range(NCORES)), trace=trace
    )
    return res.results[0]["out"], res.exec_time_ns


def kernel(x, edge_index, batch, W1, b1, W2, b2, lin_W, lin_b):
    out, _ = _run(x, edge_index, batch, W1, b1, W2, b2, lin_W, lin_b)
    return out



# revision 4
# speedup vs baseline: 1.5137x; 1.5137x over previous
"""GCN classifier (2x GCNConv + mean-pool + linear) on 8 Trainium2 NeuronCores.

Strategy:
  - Destination-node sharding: core c owns nodes [6250c, 6250(c+1)).
  - Self-loops appended as explicit edges; edges sorted by (dest block,
    table half, position).
  - y1 = dinv * (x @ W1) computed replicated on every core (bf16 table).
  - Aggregation: per (dest block, table half), one custom dma_gather pulls all
    source rows (bf16, 256B rows) into chunk layout; per 128-edge chunk a
    one-hot selection matrix (DVE is_equal, bf16) and a PE matmul accumulate
    into f32 PSUM.
  - h1 -> xw2 -> y2 slice per dest block; y2 slices exchanged via chunked
    AllGather collectives overlapped with layer-1 aggregation.
  - Mean-pool via selection matmuls into persistent PSUM accumulators,
    AllReduce of per-graph partial sums/counts, final linear on every core.
"""
import numpy as np

import concourse.bacc as bacc
import concourse.bass as bass
import concourse.mybir as mybir
import concourse.tile as tile
from concourse import bass_utils

# problem dims (hardcoded per contract)
N = 50000
E = 600000
IN_CH = 256
HID = 128
NUM_CLASSES = 2
G = 256
NCORES = 8

SLICE = N // NCORES          # 6250 nodes per core
NB = (SLICE + 127) // 128    # 49 dest blocks per core
PB = NB * 128                # 6272 padded rows per core
PADN = PB * NCORES           # 50176 padded table rows
HALF = PADN // 2             # 25088 rows per gather-table half (int16 range)
CHUNKS = [7, 7, 7, 4, 3, 7, 7, 7]   # dest blocks per allgather chunk
HALF2 = 25600                # layer-2 half boundary (chunks 0-3)
_SB = np.concatenate([[0], np.cumsum(CHUNKS)])       # chunk -> first block
_CO = np.concatenate([[0], np.cumsum([8 * 128 * w for w in CHUNKS])])  # chunk -> table row
_J_OF_B = np.repeat(np.arange(len(CHUNKS)), CHUNKS)  # block -> chunk
MAXC = 8                     # dma_gather cap: num_idxs <= 1024

dt = mybir.dt

_cache = {}


def _pos1(u):
    """node id -> row in core-major padded table (y1 layout)."""
    return PB * (u // SLICE) + (u % SLICE)


def _pos2(u):
    """node id -> row in chunk-major padded table (y2 allgather layout)."""
    c = u // SLICE
    l = u % SLICE
    b = l // 128
    p = l % 128
    j = _J_OF_B[b]
    w = np.asarray(CHUNKS)[j]
    return _CO[j] + c * (w * 128) + (b - _SB[j]) * 128 + p


def _wrap_idx(flat):
    """edge-slot-ordered positions [n] -> dma_gather wrapped layout [128, n//16]."""
    n = flat.shape[0]
    cols = n // 16
    out = np.empty((128, cols), np.int16)
    block = flat.reshape(cols, 16).T  # [16, cols]
    for g in range(8):
        out[g * 16 : (g + 1) * 16] = block
    return out


def _host_prep(x, edge_index, batch):
    x = np.asarray(x, np.float32)
    ei = np.asarray(edge_index)
    batch_np = np.asarray(batch)

    src = np.concatenate([ei[0], np.arange(N, dtype=ei.dtype)]).astype(np.int64)
    dst = np.concatenate([ei[1], np.arange(N, dtype=ei.dtype)]).astype(np.int64)
    deg = np.bincount(dst, minlength=N).astype(np.float32)  # >= 1 (self-loops)

    # per-core edges with per-layer (block, half) grouping
    layers = {1: _pos1, 2: _pos2}
    ecore = {}
    counts = {l: np.zeros((NCORES, NB, 2), np.int64) for l in layers}
    for c in range(NCORES):
        m = (dst // SLICE) == c
        es, ed = src[m], dst[m]
        ld = ed - SLICE * c
        ecore[c] = {}
        for l, posf in layers.items():
            hb = HALF if l == 1 else HALF2
            pos = posf(es)
            half = (pos >= hb).astype(np.int64)
            order = np.lexsort((pos, half, ld // 128))
            p_s, h_s, ld_s = pos[order], half[order], ld[order]
            b_s = ld_s // 128
            for b in range(NB):
                for h in (0, 1):
                    sel = (b_s == b) & (h_s == h)
                    counts[l][c, b, h] = sel.sum()
            ecore[c][l] = (p_s, h_s, b_s, ld_s % 128)

    # SPMD-uniform gather sizes per (layer, block, half): %16 granular
    NI2 = {}
    K2 = {}
    for l in layers:
        mx = counts[l].max(axis=0)  # [NB, 2]
        ni = np.maximum((mx + 15) // 16 * 16, 16)
        assert ni.max() <= 1024, ni.max()
        NI2[l] = ni.astype(np.int64)
        K2[l] = ((ni + 127) // 128).astype(np.int64)

    idx_np = {}
    colloc_np = {}
    for l in layers:
        C2 = int(K2[l].sum())
        ICOLS = int(NI2[l].sum()) // 16
        idx_np[l] = np.zeros((NCORES, 128, ICOLS), np.int16)
        colloc_np[l] = np.full((NCORES, 128, C2), -1.0, np.float32)
        hb = HALF if l == 1 else HALF2
        for c in range(NCORES):
            p_s, h_s, b_s, lp_s = ecore[c][l]
            col = 0
            icol = 0
            for h in (0, 1):
                for b in range(NB):
                    K = int(K2[l][b, h])
                    ni = int(NI2[l][b, h])
                    sel = (b_s == b) & (h_s == h)
                    k = int(sel.sum())
                    p_pad = np.zeros(ni, np.int64)
                    c_pad = np.full(K * 128, -1.0, np.float32)
                    p_pad[:k] = p_s[sel] - h * hb
                    c_pad[:k] = lp_s[sel]
                    idx_np[l][c, :, icol : icol + ni // 16] = _wrap_idx(
                        p_pad.astype(np.int16)
                    )
                    colloc_np[l][c, :, col : col + K] = c_pad.reshape(K, 128).T
                    col += K
                    icol += ni // 16
        colloc_np[l] = colloc_np[l].astype(np.float32)

    # degrees in block layout
    degp_slices = []
    degs_core = []
    batch_core = []
    for c in range(NCORES):
        dpad = np.ones(PB, np.float32)
        dpad[:SLICE] = deg[c * SLICE : (c + 1) * SLICE]
        degp_slices.append(dpad)
        degs_core.append(dpad.reshape(NB, 128).T.copy())
        bpad = np.full(PB, -1.0, np.float32)
        bpad[:SLICE] = batch_np[c * SLICE : (c + 1) * SLICE].astype(np.float32)
        batch_core.append(bpad.reshape(NB, 128).T.copy())
    degp = np.concatenate(degp_slices).reshape(NCORES * NB, 128).T.copy()

    # padded x (row-major, layer-1 gather table) + per-edge dinv[src]
    xp = np.zeros((PADN, IN_CH), np.float32)
    xp[_pos1(np.arange(N))] = x
    dinv_all = 1.0 / np.sqrt(deg)
    dsrc_np = np.zeros((NCORES, 128, int(K2[1].sum())), np.float32)
    for c in range(NCORES):
        p_s, h_s, b_s, lp_s = ecore[c][1]
        col = 0
        for h in (0, 1):
            for b in range(NB):
                K = int(K2[1][b, h])
                sel = (b_s == b) & (h_s == h)
                k = int(sel.sum())
                d_pad = np.zeros(K * 128, np.float32)
                pv = p_s[sel]
                d_pad[:k] = dinv_all[(pv // PB) * SLICE + (pv % PB)]
                dsrc_np[c][:, col : col + K] = d_pad.reshape(K, 128).T
                col += K
    return {
        "xp": xp,
        "dsrc1": dsrc_np,
        "xT": np.ascontiguousarray(xp.T),
        "degp": degp,
        "degs": degs_core,
        "batch": batch_core,
        "idx": idx_np,
        "colloc": colloc_np,
        "K2": K2,
        "NI2": NI2,
    }


def _build_program(K2, NI2):
    nc = bacc.Bacc(
        "TRN2",
        target_bir_lowering=False,
        debug=False,
        num_devices=NCORES,
        num_swdge_queues=4,
    )
    f32, bf16, i16 = dt.float32, dt.bfloat16, dt.int16
    C2 = {l: int(K2[l].sum()) for l in (1, 2)}
    IC = {l: int(NI2[l].sum()) // 16 for l in (1, 2)}

    # ---- I/O ----
    xp_d = nc.dram_tensor("xp", [PADN, IN_CH], bf16, kind="ExternalInput")
    dsrc1_d = nc.dram_tensor("dsrc1", [128, C2[1]], f32, kind="ExternalInput")
    w1_d = nc.dram_tensor("W1", [IN_CH, HID], bf16, kind="ExternalInput")
    w2_d = nc.dram_tensor("W2", [HID, HID], bf16, kind="ExternalInput")
    b1r_d = nc.dram_tensor("b1r", [128, HID], f32, kind="ExternalInput")
    b2r_d = nc.dram_tensor("b2r", [128, HID], f32, kind="ExternalInput")
    linw_d = nc.dram_tensor("linW", [HID, NUM_CLASSES], bf16, kind="ExternalInput")
    linbr_d = nc.dram_tensor("linbr", [128, NUM_CLASSES], f32, kind="ExternalInput")
    iota128_d = nc.dram_tensor("iota128", [128, 128], bf16, kind="ExternalInput")
    iota256_d = nc.dram_tensor("iota256", [128, G], bf16, kind="ExternalInput")
    identb_d = nc.dram_tensor("identb", [128, 128], bf16, kind="ExternalInput")
    degp_d = nc.dram_tensor("degp", [128, NCORES * NB], f32, kind="ExternalInput")
    degs_d = nc.dram_tensor("degs", [128, NB], f32, kind="ExternalInput")
    batch_d = nc.dram_tensor("batch", [128, NB], bf16, kind="ExternalInput")
    idx1_d = nc.dram_tensor("idx1", [128, IC[1]], i16, kind="ExternalInput")
    idx2_d = nc.dram_tensor("idx2", [128, IC[2]], i16, kind="ExternalInput")
    col1_d = nc.dram_tensor("col1", [128, C2[1]], bf16, kind="ExternalInput")
    col2_d = nc.dram_tensor("col2", [128, C2[2]], bf16, kind="ExternalInput")
    out_d = nc.dram_tensor("out", [G, NUM_CLASSES], f32, kind="ExternalOutput")

    # ---- internal DRAM ----
    y2slice = nc.dram_tensor("y2slice", [PB, HID], bf16, kind="Internal")
    y2a = nc.dram_tensor("y2a", [HALF2, HID], bf16, kind="Internal", addr_space="Shared")
    y2b = nc.dram_tensor("y2b", [PADN - HALF2, HID], bf16, kind="Internal", addr_space="Shared")
    pool_in = nc.dram_tensor("pool_in", [G, HID + 1], f32, kind="Internal")
    pool_out = nc.dram_tensor(
        "pool_out", [G, HID + 1], f32, kind="Internal", addr_space="Shared"
    )
    dum_d = nc.dram_tensor("dumt", [16, HID], bf16, kind="ExternalInput")

    AG_GROUPS = [list(range(NCORES))]

    with tile.TileContext(nc) as tc:
        with (
            tc.tile_pool(name="consts", bufs=1) as cons,
            tc.tile_pool(name="xtiles", bufs=3) as xtl,
            tc.tile_pool(name="work", bufs=4) as work,
            tc.tile_pool(name="idxt", bufs=4) as idxt,
            tc.tile_pool(name="msgs", bufs=12) as msgs,
            tc.tile_pool(name="sels", bufs=32) as sels,
            tc.tile_pool(name="spill", bufs=1) as spl,
            tc.tile_pool(name="psum", bufs=2, space="PSUM") as pst,
            tc.tile_pool(name="psumx", bufs=1, space="PSUM") as psx,
            tc.tile_pool(name="psump", bufs=1, space="PSUM") as psp,
        ):
            # ---- constants ----
            w1a = cons.tile([128, HID], bf16, tag="w1a")
            w1b = cons.tile([128, HID], bf16, tag="w1b")
            w2 = cons.tile([128, HID], bf16, tag="w2")
            b1r = cons.tile([128, HID], f32, tag="b1r")
            b2r = cons.tile([128, HID], f32, tag="b2r")
            linw = cons.tile([HID, NUM_CLASSES], bf16, tag="linw")
            linbr = cons.tile([128, NUM_CLASSES], f32, tag="linbr")
            iota128 = cons.tile([128, 128], bf16, tag="iota128")
            iota256 = cons.tile([128, G], bf16, tag="iota256")
            identb = cons.tile([128, 128], bf16, tag="identb")
            degp = cons.tile([128, NCORES * NB], f32, tag="degp")
            degs = cons.tile([128, NB], f32, tag="degs")
            batcht = cons.tile([128, NB], bf16, tag="batcht")
            col1 = cons.tile([128, C2[1]], bf16, tag="col1")
            col2 = cons.tile([128, C2[2]], bf16, tag="col2")
            dsrc1 = cons.tile([128, C2[1]], f32, tag="dsrc1")
            idx1 = cons.tile([128, IC[1]], i16, tag="idx1")
            idx2 = cons.tile([128, IC[2]], i16, tag="idx2")
            dinv_g = cons.tile([128, NCORES * NB], f32, tag="dinv_g")
            dinv_s = cons.tile([128, NB], f32, tag="dinv_s")

            for t, d in (
                (w1a, w1_d[0:128, :]), (w1b, w1_d[128:256, :]), (w2, w2_d[:]),
                (b1r, b1r_d[:]), (b2r, b2r_d[:]), (linw, linw_d[:]),
                (linbr, linbr_d[:]), (iota128, iota128_d[:]),
                (iota256, iota256_d[:]), (identb, identb_d[:]),
                (degp, degp_d[:]), (degs, degs_d[:]), (batcht, batch_d[:]),
                (col1, col1_d[:]), (col2, col2_d[:]),
                (idx1, idx1_d[:]), (idx2, idx2_d[:]), (dsrc1, dsrc1_d[:]),
            ):
                nc.sync.dma_start(t[:], d)

            # tiny dummy gather up front so the GPSIMD "mlp" library reload
            # overlaps phase A instead of stalling the first real gather
            # (one per SWDGE queue so every Q7 pair is warm)
            dumi = cons.tile([128, 1], i16, tag="dumi")
            nc.vector.memset(dumi[:], 0)
            for q in range(4):
                dumm = cons.tile([128, HID], bf16, tag=f"dumm{q}")
                nc.gpsimd.dma_gather(
                    out_ap=dumm[:].rearrange("p (c e) -> p c e", e=HID),
                    in_ap=dum_d[:],
                    idxs_ap=dumi[:],
                    num_idxs=16,
                    num_idxs_reg=16,
                    elem_size=HID,
                    queue_num=q,
                )

            # dinv = 1/sqrt(deg)
            nc.scalar.activation(dinv_g[:], degp[:], mybir.ActivationFunctionType.Sqrt)
            nc.vector.reciprocal(dinv_g[:], dinv_g[:])
            nc.scalar.activation(dinv_s[:], degs[:], mybir.ActivationFunctionType.Sqrt)
            nc.vector.reciprocal(dinv_s[:], dinv_s[:])

            # ---- phase A: y1 = dinv * (x @ W1), full table, replicated ----


            # warm all msg pool slots so never-gathered tail rows are finite
            for _ in range(12):
                wt = msgs.tile([128, MAXC * IN_CH], bf16, tag="msg")
                nc.vector.memset(wt[:], 0.0)

            # ---- phases B (layer 1) and C (layer 2) ----
            for layer in (1, 2):
                K = K2[layer]
                NI = NI2[layer]
                idxt_t = idx1 if layer == 1 else idx2
                coll = col1 if layer == 1 else col2
                EW = IN_CH if layer == 1 else HID
                tables = (None, None) if layer == 1 else (y2a, y2b)
                brep = b1r if layer == 1 else b2r
                ibase = 0
                cbase = 0
                spills = {}
                for h in (0, 1):
                  table_h = tables[h]
                  for b in range(NB):
                    Kh = int(K[b, h])
                    ni = int(NI[b, h])
                    ps = pst.tile([128, EW], f32, tag="agg", space="PSUM")
                    mt = msgs.tile([128, MAXC * IN_CH], bf16, tag="msg")
                    nc.gpsimd.dma_gather(
                        out_ap=mt[:, : Kh * EW].rearrange("p (c e) -> p c e", e=EW),
                        in_ap=(xp_d[h * HALF : (h + 1) * HALF, :] if layer == 1 else table_h[:]),
                        idxs_ap=idxt_t[:, ibase : ibase + ni // 16],
                        num_idxs=ni,
                        num_idxs_reg=ni,
                        elem_size=EW,
                        queue_num=(h * NB + b) % 4,
                    )
                    for j in range(Kh):
                        S = sels.tile([128, 128], bf16, tag="sel")
                        nc.vector.tensor_tensor(
                            out=S[:],
                            in0=coll[:, cbase + j : cbase + j + 1].to_broadcast(
                                [128, 128]
                            ),
                            in1=iota128[:],
                            op=mybir.AluOpType.is_equal,
                        )
                        if layer == 1:
                            nc.vector.tensor_tensor(
                                out=S[:],
                                in0=S[:],
                                in1=dsrc1[:, cbase + j : cbase + j + 1].to_broadcast(
                                    [128, 128]
                                ),
                                op=mybir.AluOpType.mult,
                            )
                        nc.tensor.matmul(
                            ps[:],
                            S[:],
                            mt[:, j * EW : (j + 1) * EW],
                            start=(j == 0),
                            stop=(j == Kh - 1),
                        )
                    ibase += ni // 16
                    cbase += Kh
                    if h == 0:
                        sp = spl.tile([128, EW], f32, tag=f"sp{b}")
                        nc.vector.tensor_copy(out=sp[:], in_=ps[:])
                        spills[b] = sp
                        continue
                    # epilogue: h = relu(dinv * (agg0 + agg1)[@W1] + b)
                    if layer == 1:
                        axf = work.tile([128, IN_CH], bf16, tag="axf")
                        nc.vector.tensor_tensor(
                            out=axf[:], in0=ps[:], in1=spills[b][:],
                            op=mybir.AluOpType.add,
                        )
                        agg1 = psx.tile([128, HID], f32, tag="xw1o", space="PSUM")
                        for q in range(2):
                            ptq = psx.tile([128, 128], bf16, tag="trans", space="PSUM")
                            nc.tensor.transpose(
                                out=ptq[:], in_=axf[:, q * 128 : (q + 1) * 128],
                                identity=identb[:],
                            )
                            tq = work.tile([128, 128], bf16, tag="tq")
                            nc.vector.tensor_copy(out=tq[:], in_=ptq[:])
                            nc.tensor.matmul(
                                agg1[:], tq[:], w1a[:] if q == 0 else w1b[:],
                                start=(q == 0), stop=(q == 1),
                            )
                        accv = agg1
                    else:
                        accv = work.tile([128, HID], f32, tag="hf0")
                        nc.vector.tensor_tensor(
                            out=accv[:], in0=ps[:], in1=spills[b][:],
                            op=mybir.AluOpType.add,
                        )
                    hf = work.tile([128, HID], f32, tag="hf")
                    nc.vector.tensor_tensor(
                        out=hf[:],
                        in0=accv[:],
                        in1=dinv_s[:, b : b + 1].to_broadcast([128, HID]),
                        op=mybir.AluOpType.mult,
                    )
                    nc.vector.tensor_tensor(
                        out=hf[:], in0=hf[:], in1=brep[:], op=mybir.AluOpType.add
                    )
                    hb = work.tile([128, HID], bf16, tag="hb")
                    nc.scalar.activation(
                        hb[:], hf[:], mybir.ActivationFunctionType.Relu
                    )
                    if layer == 1:
                        # xw2 = h1 @ W2 ; y2 = dinv * xw2
                        pt = psx.tile([128, 128], bf16, tag="trans", space="PSUM")
                        nc.tensor.transpose(out=pt[:], in_=hb[:], identity=identb[:])
                        hT = work.tile([128, 128], bf16, tag="hT")
                        nc.vector.tensor_copy(out=hT[:], in_=pt[:])
                        p2 = psx.tile([128, HID], f32, tag="xw2", space="PSUM")
                        nc.tensor.matmul(p2[:], hT[:], w2[:], start=True, stop=True)
                        y2w = work.tile([128, HID], bf16, tag="y2w")
                        nc.vector.tensor_tensor(
                            out=y2w[:],
                            in0=p2[:],
                            in1=dinv_s[:, b : b + 1].to_broadcast([128, HID]),
                            op=mybir.AluOpType.mult,
                        )
                        nc.sync.dma_start(y2slice[b * 128 : (b + 1) * 128, :], y2w[:])
                        if b + 1 in _SB[1:]:
                            j = int(np.searchsorted(_SB, b + 1)) - 1
                            r0 = int(_SB[j]) * 128
                            rn = CHUNKS[j] * 128
                            o0 = int(_CO[j])
                            ytgt, yo = (y2a, o0) if j <= 3 else (y2b, o0 - HALF2)
                            nc.gpsimd.collective_compute(
                                "AllGather",
                                mybir.AluOpType.bypass,
                                replica_groups=AG_GROUPS,
                                ins=[y2slice[r0 : r0 + rn, :].opt()],
                                outs=[ytgt[yo : yo + NCORES * rn, :].opt()],
                            )
                    else:
                        # pooling: append ones column, selection matmuls
                        ho = work.tile([128, HID + 1], bf16, tag="hones")
                        nc.vector.tensor_copy(out=ho[:, :HID], in_=hb[:])
                        nc.vector.memset(ho[:, HID : HID + 1], 1.0)
                        Sp = work.tile([128, G], bf16, tag="spool")
                        nc.vector.tensor_tensor(
                            out=Sp[:],
                            in0=batcht[:, b : b + 1].to_broadcast([128, G]),
                            in1=iota256[:],
                            op=mybir.AluOpType.is_equal,
                        )
                        if b == 0:
                            ppA = psp.tile([128, HID + 1], f32, tag="poolA", space="PSUM")
                            ppB = psp.tile([128, HID + 1], f32, tag="poolB", space="PSUM")
                        nc.tensor.matmul(
                            ppA[:], Sp[:, 0:128], ho[:], start=(b == 0), stop=(b == NB - 1)
                        )
                        nc.tensor.matmul(
                            ppB[:], Sp[:, 128:256], ho[:], start=(b == 0), stop=(b == NB - 1)
                        )

            # ---- phase D: reduce partial sums, final linear ----
            sA = work.tile([128, HID + 1], f32, tag="sA")
            sB = work.tile([128, HID + 1], f32, tag="sB")
            nc.vector.tensor_copy(out=sA[:], in_=ppA[:])
            nc.vector.tensor_copy(out=sB[:], in_=ppB[:])
            nc.sync.dma_start(pool_in[0:128, :], sA[:])
            nc.sync.dma_start(pool_in[128:256, :], sB[:])
            nc.gpsimd.collective_compute(
                "AllReduce",
                mybir.AluOpType.add,
                replica_groups=AG_GROUPS,
                ins=[pool_in[:].opt()],
                outs=[pool_out[:].opt()],
            )
            for half in range(2):
                s = work.tile([128, HID + 1], f32, tag="sred")
                nc.sync.dma_start(s[:], pool_out[half * 128 : (half + 1) * 128, :])
                cnt = work.tile([128, 1], f32, tag="cnt")
                nc.vector.tensor_scalar_max(cnt[:], s[:, HID : HID + 1], 1.0)
                rc = work.tile([128, 1], f32, tag="rc")
                nc.vector.reciprocal(rc[:], cnt[:])
                sbt = work.tile([128, HID], bf16, tag="sbt")
                nc.vector.tensor_copy(out=sbt[:], in_=s[:, 0:HID])
                pt = psx.tile([128, 128], bf16, tag="trans", space="PSUM")
                nc.tensor.transpose(out=pt[:], in_=sbt[:], identity=identb[:])
                sT = work.tile([128, 128], bf16, tag="sT")
                nc.vector.tensor_copy(out=sT[:], in_=pt[:])
                po = psx.tile([128, NUM_CLASSES], f32, tag="xw1o", space="PSUM")
                nc.tensor.matmul(po[:], sT[:], linw[:], start=True, stop=True)
                ob = work.tile([128, NUM_CLASSES], f32, tag="ob")
                nc.vector.tensor_tensor(
                    out=ob[:],
                    in0=po[:],
                    in1=rc[:].to_broadcast([128, NUM_CLASSES]),
                    op=mybir.AluOpType.mult,
                )
                nc.vector.tensor_tensor(
                    out=ob[:], in0=ob[:], in1=linbr[:], op=mybir.AluOpType.add
                )
                nc.sync.dma_start(out_d[half * 128 : (half + 1) * 128, :], ob[:])

    nc.compile()
    return nc


def _get_program(prep):
    key = tuple(tuple(map(tuple, prep["NI2"][l])) for l in (1, 2))
    if key not in _cache:
        _cache[key] = _build_program(prep["K2"], prep["NI2"])
    return _cache[key]


def _to_bf16(a):
    import ml_dtypes

    return np.asarray(a, np.float32).astype(ml_dtypes.bfloat16)


def _run(x, edge_index, batch, W1, b1, W2, b2, lin_W, lin_b, trace=False):
    prep = _host_prep(x, edge_index, batch)
    nc = _get_program(prep)

    b1r = np.tile(np.asarray(b1, np.float32)[None, :], (128, 1))
    b2r = np.tile(np.asarray(b2, np.float32)[None, :], (128, 1))
    linbr = np.tile(np.asarray(lin_b, np.float32)[None, :], (128, 1))
    iota128 = _to_bf16(np.tile(np.arange(128, dtype=np.float32)[None, :], (128, 1)))
    iota256 = _to_bf16(np.tile(np.arange(G, dtype=np.float32)[None, :], (128, 1)))
    identb = _to_bf16(np.eye(128, dtype=np.float32))
    xTb = _to_bf16(prep["xT"])
    xpb = _to_bf16(prep["xp"])
    W1b = _to_bf16(W1)
    W2b = _to_bf16(W2)
    linwb = _to_bf16(lin_W)

    in_maps = []
    for c in range(NCORES):
        in_maps.append(
            {
                "xp": xpb,
                "dsrc1": prep["dsrc1"][c],
                "xT": xTb,
                "W1": W1b,
                "W2": W2b,
                "b1r": b1r,
                "b2r": b2r,
                "linW": linwb,
                "linbr": linbr,
                "iota128": iota128,
                "iota256": iota256,
                "identb": identb,
                "degp": prep["degp"],
                "degs": prep["degs"][c],
                "batch": _to_bf16(prep["batch"][c]),
                "dumt": _to_bf16(np.zeros((16, HID), np.float32)),
                "idx1": np.ascontiguousarray(prep["idx"][1][c]),
                "idx2": np.ascontiguousarray(prep["idx"][2][c]),
                "col1": _to_bf16(prep["colloc"][1][c]),
                "col2": _to_bf16(prep["colloc"][2][c]),
            }
        )

    res = bass_utils.run_bass_kernel_spmd(
        nc, in_maps, core_ids=list(range(NCORES)), trace=trace
    )
    return res.results[0]["out"], res.exec_time_ns


def kernel(x, edge_index, batch, W1, b1, W2, b2, lin_W, lin_b):
    out, _ = _run(x, edge_index, batch, W1, b1, W2, b2, lin_W, lin_b)
    return out

